# revision 1
# baseline (speedup 1.0000x reference)
"""Multi-head self-attention (b=4, n=2048, d=512, h=8, dh=64) on 8 trn2 cores.

Sharding v2: core c -> (batch b = c//2, head half hh = c%2). Each core computes
4 heads over the FULL sequence; the two cores of a batch produce partial
outputs (their heads' slice of the output projection) that the host SUMS,
adding the bias. No K/V recompute, no sequence split.

Per-core kernel (matmuls bf16, f32 PSUM accumulation):
  xT  [128, 4, 2048]   x[b].T chunked (row r of x.T at [r%128, r//128])
  Wq/Wk/Wv [128, 4, 256]  this core's 4 heads' columns, chunked
  Wo  [128, 2, 512]    this core's 256 rows of Wo, chunked
  QT = Wq^T x^T  [128, 2, 2048]  chunk i holds heads (2i, 2i+1) stacked 64+64
  KT likewise; V -> Vaug [128 kv, 16 j, 4 h, 65] with ones column (denom row)
  S pair: two K=64 matmuls row-tiled at (0,0)/(64,0) run CONCURRENTLY in the
    PE array -> 2-slot PSUM pack [128, 2, 512] (adjacent banks)
  exp: alternates per j between ACT (true exp) and DVE (Schraudolph int16
    bitcast exp, ~2% rms) so consecutive packs' exps overlap on two engines
  AV: per (head, n-block) one PSUM bank accumulates all 16 j matmuls
    ([Vh|1]^T E), row 64 = softmax denominator. No DVE adds.
  normalize: copy+recip (DVE) -> partition_broadcast (GpSimd) -> mul (DVE)
  out = OT^T Wo (partial; host adds pair partner + bias)
"""

import sys

sys.path.insert(0, "/opt/trn_rl_repo")

from contextlib import ExitStack

import ml_dtypes
import numpy as np

import concourse.bass as bass
import concourse.tile as tile
from concourse import bacc, mybir
from concourse.bass import ts, ds
from concourse.bass_utils import run_bass_kernel_spmd

BF16 = mybir.dt.bfloat16
F32 = mybir.dt.float32
I16 = mybir.dt.int16

D = 512         # model dim
HL = 4          # heads per core
DH = 64
N = 2048        # full sequence per core
P = 128
KO = 4          # xT chunks of model dim
JT = 16         # kv tiles of 128
NB = 4          # q blocks of 512
SCALE = DH ** -0.5
# Schraudolph exp: bf16 bitcast of int16(x*128/ln2 + (127<<7) - C)
SCH_A = float(128.0 / np.log(2.0) * SCALE)
SCH_B = float(127 * 128 - 4)

# which j-slots of each 16-slot phase the DVE handles exp for (rest: ACT)
DVE_EXP_SLOTS = {1, 3, 5, 7, 9, 11, 13, 15}


def build_nc(finalize=True, dbg=False):
    nc = bacc.Bacc("TRN2", target_bir_lowering=False)

    xT_d = nc.dram_tensor("xT", [P, KO, N], BF16, kind="ExternalInput")
    Wq_d = nc.dram_tensor("Wq", [P, KO, 256], BF16, kind="ExternalInput")
    Wk_d = nc.dram_tensor("Wk", [P, KO, 256], BF16, kind="ExternalInput")
    Wv_d = nc.dram_tensor("Wv", [P, KO, 256], BF16, kind="ExternalInput")
    Wo_d = nc.dram_tensor("Wo", [P, 2, D], BF16, kind="ExternalInput")
    out_d = nc.dram_tensor("out", [N, D], F32, kind="ExternalOutput")
    if dbg:
        QT_o = nc.dram_tensor("QT_o", [P, 2, N], BF16, kind="ExternalOutput")
        KT_o = nc.dram_tensor("KT_o", [P, 2, N], BF16, kind="ExternalOutput")
        Va_o = nc.dram_tensor("Va_o", [P, JT, HL, DH + 1], BF16,
                              kind="ExternalOutput")
        OT_o = nc.dram_tensor("OT_o", [P, 2, N], BF16, kind="ExternalOutput")
        S_o = nc.dram_tensor("S_o", [P, 2, 512], F32, kind="ExternalOutput")
        E_o = nc.dram_tensor("E_o", [P, 2, 512], BF16, kind="ExternalOutput")
        E5_o = nc.dram_tensor("E5_o", [P, 2, 512], BF16, kind="ExternalOutput")
        AV_o = nc.dram_tensor("AV_o", [DH + 1, 512], F32, kind="ExternalOutput")

    with tile.TileContext(nc) as tc, ExitStack() as ctx:
        consts = ctx.enter_context(tc.tile_pool(name="consts", bufs=1))
        # S score pairs: [128, 2, 512] = 2 PSUM banks each
        ps = ctx.enter_context(tc.tile_pool(name="ps", bufs=2, space="PSUM"))
        # AV accumulators only -- their 16-matmul accumulation groups stay
        # open across many slots, so nothing else may share these banks
        avp = ctx.enter_context(tc.tile_pool(name="avp", bufs=2, space="PSUM"))
        # projection / O-proj / warmup scratch (one 2-bank buffer)
        projp = ctx.enter_context(tc.tile_pool(name="projp", bufs=1, space="PSUM"))
        expp = ctx.enter_context(tc.tile_pool(name="expp", bufs=10))
        small = ctx.enter_context(tc.tile_pool(name="small", bufs=4))
        outp = ctx.enter_context(tc.tile_pool(name="outp", bufs=2))

        # ---- persistent SBUF tensors ----
        xT_sb = consts.tile([P, KO, N], BF16, tag="xT")
        Wq_sb = consts.tile([P, KO, 256], BF16, tag="Wq")
        Wk_sb = consts.tile([P, KO, 256], BF16, tag="Wk")
        Wv_sb = consts.tile([P, KO, 256], BF16, tag="Wv")
        Wo_sb = consts.tile([P, 2, D], BF16, tag="Wo")
        QT_sb = consts.tile([P, 2, N], BF16, tag="QT")
        KT_sb = consts.tile([P, 2, N], BF16, tag="KT")
        Vaug_sb = consts.tile([P, JT, HL, DH + 1], BF16, tag="Vaug")
        OT_sb = consts.tile([P, 2, N], BF16, tag="OT")

        # input DMAs: xT in column blocks so compute can start early
        nc.sync.dma_start(Wq_sb[:], Wq_d[:])
        nc.sync.dma_start(xT_sb[:, :, 0:512], xT_d[:, :, 0:512])
        nc.sync.dma_start(Wk_sb[:], Wk_d[:])
        nc.sync.dma_start(xT_sb[:, :, 512:1024], xT_d[:, :, 512:1024])
        nc.sync.dma_start(Wv_sb[:], Wv_d[:])
        nc.sync.dma_start(xT_sb[:, :, 1024:1536], xT_d[:, :, 1024:1536])
        nc.sync.dma_start(Wo_sb[:], Wo_d[:])
        nc.sync.dma_start(xT_sb[:, :, 1536:2048], xT_d[:, :, 1536:2048])

        nc.vector.memset(Vaug_sb[:, :, :, DH : DH + 1], 1.0)

        # spin the PE so HAM unthrottles before the first real matmuls
        junk = small.tile([64, 64], BF16, tag="junk")
        nc.vector.memset(junk[:], 0.0)
        wp = projp.tile([P, 2, 512], F32, tag="projp", name="wp")
        for _ in range(64):
            nc.tensor.matmul(wp[0:64, 0, 0:64], lhsT=junk[:], rhs=junk[:],
                             start=True, stop=True)
        # touch the exp table early so ACT_TABLE_LOAD overlaps the DMAs
        warm = small.tile([1, 8], F32, tag="warm")
        nc.scalar.activation(warm[:], junk[0:1, 0:8],
                             mybir.ActivationFunctionType.Exp)

        # ---- projection units (two 512-blocks per unit, k-outer so the
        # stationary weight chunk serves both blocks back-to-back) ----
        def proj_unit(W_sb, T_sb, o, np_, dve_cast=False):
            pp = projp.tile([P, 2, 512], F32, tag="projp",
                            name=f"pp{id(W_sb)%97}_{o}_{np_}")
            for k in range(KO):
                for m in range(2):
                    nc.tensor.matmul(
                        pp[:, m, :],
                        lhsT=W_sb[:, k, ts(o, P)],
                        rhs=xT_sb[:, k, ds(np_ * 1024 + m * 512, 512)],
                        start=(k == 0),
                        stop=(k == KO - 1),
                    )
            if dve_cast:
                # prelude units: DVE is idle and ACT is busy table-loading
                nc.vector.tensor_copy(T_sb[:, o, ds(np_ * 1024, 1024)], pp[:])
            else:
                nc.scalar.activation(T_sb[:, o, ds(np_ * 1024, 1024)], pp[:],
                                     mybir.ActivationFunctionType.Copy)

        def q_unit(o, np_, dve_cast=False):
            proj_unit(Wq_sb, QT_sb, o, np_, dve_cast)

        def k_unit(o, np_, dve_cast=False):
            proj_unit(Wk_sb, KT_sb, o, np_, dve_cast)

        def v_unit(jj):
            # two kv j-tiles (256 output cols each) in one PSUM bank
            vp = projp.tile([P, 2, 256], F32, tag="projp", name=f"vp{jj}")
            for m in range(2):
                for k in range(KO):
                    nc.tensor.matmul(
                        vp[:, m, :],
                        lhsT=xT_sb[:, k, ds((2 * jj + m) * P, P)],
                        rhs=Wv_sb[:, k, :],
                        start=(k == 0),
                        stop=(k == KO - 1),
                    )
            # ACT copies carry the V cast to offload the DVE
            nc.scalar.activation(
                Vaug_sb[:, 2 * jj : 2 * jj + 2, :, 0:DH],
                vp.rearrange("p m (h d) -> p m h d", h=HL),
                mybir.ActivationFunctionType.Copy,
            )

        # ---- attention stream state ----
        av_tiles = {}     # (h, n) -> psum accumulator [65, 512]
        exp_tiles = {}    # (i, n, j) -> E tile [128, 2, 512] bf16

        def s_exp(i, n, j):
            """Score pair (heads 2i, 2i+1) + exp for q block n, kv tile j.

            The two K=64 matmuls go to row groups (0,0)/(64,0) of the PE
            array and adjacent PSUM banks; they execute concurrently.
            exp alternates ACT (true exp) / DVE (Schraudolph) per j so the
            ps double-buffer WAR chain runs on two engines in parallel.
            """
            sp = ps.tile([P, 2, 512], F32, tag="ps", name=f"sp{i}_{n}_{j}")
            nc.tensor.matmul(
                sp[:, 0, :],
                lhsT=KT_sb[0:64, i, ts(j, P)],
                rhs=QT_sb[0:64, i, ts(n, 512)],
                start=True, stop=True,
                tile_position=(0, 0),
            )
            nc.tensor.matmul(
                sp[:, 1, :],
                lhsT=KT_sb[64:128, i, ts(j, P)],
                rhs=QT_sb[64:128, i, ts(n, 512)],
                start=True, stop=True,
                tile_position=(64, 0),
            )
            eb = expp.tile([P, 2, 512], BF16, tag="expS", name=f"eb{i}_{n}_{j}")
            if j in DVE_EXP_SLOTS:
                nc.vector.tensor_scalar(
                    eb[:].bitcast(I16), sp[:], SCH_A, SCH_B,
                    mybir.AluOpType.mult, mybir.AluOpType.add,
                )
            else:
                nc.scalar.activation(
                    eb[:], sp[:], mybir.ActivationFunctionType.Exp,
                    scale=SCALE,
                )
            exp_tiles[(i, n, j)] = eb
            if dbg and i == 0 and n == 0 and j == 0:
                sc = small.tile([P, 2, 512], F32, tag="sdbg")
                nc.vector.tensor_copy(sc[:], sp[:])
                nc.sync.dma_start(S_o[:], sc[:])
                nc.sync.dma_start(E_o[:], eb[:])
            if dbg and i == 0 and n == 0 and j == 5:
                nc.sync.dma_start(E5_o[:], eb[:])

        def av(i, n, j, s):
            """Accumulate [V|1]^T E for head 2i+s of pair i into PSUM."""
            eb = exp_tiles[(i, n, j)]
            h = 2 * i + s
            if j == 0:
                av_tiles[(h, n)] = avp.tile(
                    [DH + 1, 512], F32, tag="avp", name=f"av{h}_{n}"
                )
            nc.tensor.matmul(
                av_tiles[(h, n)][:],
                lhsT=Vaug_sb[:, j, h, :],
                rhs=eb[:, s, :],
                start=(j == 0),
                stop=(j == JT - 1),
                skip_group_check=True,
            )
            if s == 1:
                del exp_tiles[(i, n, j)]

        def av_finish(i, n):
            """Normalize pair i's accumulated AV for q block n into OT."""
            if dbg and i == 0 and n == 0:
                ac = small.tile([DH + 1, 512], F32, tag="avdbg")
                nc.vector.tensor_copy(ac[:], av_tiles[(0, 0)][:])
                nc.sync.dma_start(AV_o[:], ac[:])
            for s in range(2):
                h = 2 * i + s
                acc = av_tiles.pop((h, n))
                # row 64 (partition 64) must be shifted to partition 0 by a
                # copy -- reciprocal's uop can't do cross-partition bases
                rs = small.tile([1, 512], F32, tag="rs", name=f"rs{h}_{n}")
                nc.scalar.activation(rs[:], acc[DH : DH + 1, :],
                                     mybir.ActivationFunctionType.Copy)
                rc = small.tile([1, 512], F32, tag="rc", name=f"rc{h}_{n}")
                nc.vector.reciprocal_approx_fast(rc[:], rs[:])
                rb = small.tile([DH, 512], F32, tag="rb", name=f"rb{h}_{n}")
                nc.gpsimd.partition_broadcast(rb[:], rc[:])
                nc.vector.tensor_mul(
                    OT_sb[64 * s : 64 * s + DH, i, ts(n, 512)],
                    acc[0:DH, :],
                    rb[:],
                )

        out_r = out_d.rearrange("(t p) d -> p t d", p=P)

        def o_unit(t, pool=None, dve_copy=False):
            """Output projection for q tile t (128 rows) + DMA out."""
            pool = pool or projp
            op = pool.tile([P, 2, 512], F32, tag=pool is ps and "ps" or "projp",
                           name=f"op{t}")
            for c in range(2):
                nc.tensor.matmul(
                    op[:, 0, :],
                    lhsT=OT_sb[:, c, ts(t, P)],
                    rhs=Wo_sb[:, c, :],
                    start=(c == 0),
                    stop=(c == 1),
                )
            ot = outp.tile([P, D], F32, tag="out")
            if dve_copy:
                nc.vector.tensor_copy(ot[:], op[:, 0, :])
            else:
                nc.scalar.activation(ot[:], op[:, 0, :],
                                     mybir.ActivationFunctionType.Copy)
            nc.sync.dma_start(out_r[:, t, :], ot[:])

        # ---- schedule ----
        # extras[(i, n, j)] = list of thunks to emit after that slot's AV
        extras = {}

        def addx(i, n, j, fn):
            extras.setdefault((i, n, j), []).append(fn)

        # prelude computes q/k chunk 0 first n-pair; rest spread over pair 0
        # phase (0,0): V units at even slots (v_unit(jj) before AV(j=2jj)
        # at slot 2jj+AV_LAG), K chunk-0 second n-pair at slot 5
        for jj in range(8):
            addx(0, 0, 2 * jj, lambda jj=jj: v_unit(jj))
        addx(0, 0, 5, lambda: k_unit(0, 1))
        # phase (0,1): Q chunk 0 second pair, chunk 1 starts
        addx(0, 1, 1, lambda: q_unit(0, 1))
        addx(0, 1, 7, lambda: q_unit(1, 0))
        addx(0, 1, 13, lambda: k_unit(1, 0))
        # phase (0,2): chunk 1 rest
        addx(0, 2, 3, lambda: q_unit(1, 1))
        addx(0, 2, 9, lambda: k_unit(1, 1))

        # O-projection of q block n-1 during pair-1 phase n
        for n in range(1, NB):
            for t in range(4):
                # finish(1, n-1) is emitted at slot 6 of phase (1, n) under
                # the flat schedule -- o_units must come after it
                addx(1, n, 8 + 2 * t, lambda t=t, n=n: o_unit(4 * (n - 1) + t))

        AV_LAG = 7

        # prelude: minimum to start the stream
        q_unit(0, 0, dve_cast=True)
        k_unit(0, 0, dve_cast=True)

        # the stream, FLAT across phases: S pairs flow continuously through
        # phase boundaries (no S-less drain slots), with the AV stream
        # lagging AV_LAG slots behind on the flattened index. AV matmuls
        # sandwich the S pair so their LDWEIGHTS hide under preceding work.
        phases = [(i, n) for i in range(2) for n in range(NB)]
        total = len(phases) * JT
        for g in range(total + AV_LAG):
            ag = g - AV_LAG
            if ag >= 0:
                ia, na = phases[ag // JT]
                av(ia, na, ag % JT, 0)
            if g < total:
                i, n = phases[g // JT]
                s_exp(i, n, g % JT)
            if ag >= 0:
                av(ia, na, ag % JT, 1)
                if ag % JT == JT - 1:
                    av_finish(ia, na)
            if g < total:
                for fn in extras.get((i, n, g % JT), []):
                    fn()

        # tail: last O projections (ps pool is drained by now -- alternate
        # pools and copy engines to break the single-buffer WAR chains)
        for t in range(4):
            o_unit(12 + t, pool=(ps if t % 2 == 0 else projp),
                   dve_copy=(t % 2 == 1))

        if dbg:
            nc.sync.dma_start(QT_o[:], QT_sb[:])
            nc.sync.dma_start(KT_o[:], KT_sb[:])
            nc.sync.dma_start(Va_o[:], Vaug_sb[:])
            nc.sync.dma_start(OT_o[:], OT_sb[:])

    if finalize:
        nc.finalize()
    return nc


_NC_CACHE = None


def _get_nc():
    global _NC_CACHE
    if _NC_CACHE is None:
        _NC_CACHE = build_nc()
    return _NC_CACHE


def _chunked(w):
    """[512, M] -> [128, 4, M] with row r at [r % 128, r // 128]."""
    m = w.shape[1]
    return np.ascontiguousarray(
        w.reshape(w.shape[0] // P, P, m).transpose(1, 0, 2)
    )


def make_in_maps(x, Wq, Wkv, Wo, bo):
    bf = ml_dtypes.bfloat16
    Wq = np.asarray(Wq, np.float32)
    Wkv = np.asarray(Wkv, np.float32)
    Wo = np.asarray(Wo, np.float32)
    xTs = []
    for b in range(4):
        xTs.append(_chunked(np.asarray(x[b], np.float32).T).astype(bf))
    in_maps = []
    for c in range(8):
        b, hh = divmod(c, 2)
        cols = slice(hh * 256, (hh + 1) * 256)
        in_maps.append({
            "xT": xTs[b],
            "Wq": _chunked(Wq[:, cols]).astype(bf),
            "Wk": _chunked(Wkv[:, :D][:, cols]).astype(bf),
            "Wv": _chunked(Wkv[:, D:][:, cols]).astype(bf),
            "Wo": _chunked(Wo[hh * 256 : (hh + 1) * 256, :]).astype(bf),
        })
    return in_maps


def gather_out(results, x, bo):
    b_total = x.shape[0]
    bo = np.asarray(bo, np.float32)
    out = np.empty((b_total, N, D), np.float32)
    for b in range(b_total):
        out[b] = results[2 * b]["out"] + results[2 * b + 1]["out"] + bo
    return out


def kernel(x, Wq, Wkv, Wo, bo, trace=False):
    nc = _get_nc()
    in_maps = make_in_maps(x, Wq, Wkv, Wo, bo)
    res = run_bass_kernel_spmd(nc, in_maps, core_ids=list(range(8)), trace=trace)
    out = gather_out(res.results, np.asarray(x), bo)
    if trace:
        kernel.last_exec_time_ns = res.exec_time_ns
    return out


kernel.last_exec_time_ns = None



# revision 8
# speedup vs baseline: 1.1026x; 1.1026x over previous
"""Multi-head self-attention (b=4, n=2048, d=512, h=8, dh=64) on 8 trn2 cores.

Sharding: core c -> (batch b = c//2, head half hh = c%2). Each core computes
4 heads over the FULL sequence; the two cores of a batch produce partial
outputs (their heads' slice of the output projection) that the host SUMS,
adding the bias.

v3 schedule notes (vs v2 baseline):
  - ONE shared 3-buffer PSUM pool (6 banks) rotates S-score pairs AND all
    projection units, so the S stream runs 3 deep: S(g+3) only WARs on
    exp(g), letting the two exp engines (ACT true-exp / DVE Schraudolph)
    overlap fully instead of serializing the slot cadence.
  - AV accumulators for the head pair live in ONE [65, 2, 512] PSUM tile
    (2 banks).  Normalize is pair-batched: reciprocal_approx_fast reads the
    whole tile straight from PSUM (rows 0..63 produce garbage recips that
    are never read - only row 64, the ones-row denominator, is used), so
    the old per-head row-64 ACT extraction copies are gone.
  - The denominator broadcast runs on GpSimd; the two OT multiplies are
    emitted a few slots later so they never head-of-line block DVE exps.
  - Input DMA is reordered (Wq, xT[0:1024], Wk, ...) so the Q/K prelude
    projections start ~5us in instead of waiting for the full 2.8MB.
  - AV lags the S stream by 6 slots; the tail drains in ~10us.
"""

import sys

sys.path.insert(0, "/opt/trn_rl_repo")

from contextlib import ExitStack

import ml_dtypes
import numpy as np

import concourse.bass as bass
import concourse.tile as tile
from concourse import bacc, mybir
from concourse.bass import ts, ds
from concourse.bass_utils import run_bass_kernel_spmd

BF16 = mybir.dt.bfloat16
F32 = mybir.dt.float32
I16 = mybir.dt.int16

D = 512         # model dim
HL = 4          # heads per core
DH = 64
N = 2048        # full sequence per core
P = 128
KO = 4          # xT chunks of model dim
JT = 16         # kv tiles of 128
NB = 4          # q blocks of 512
SCALE = DH ** -0.5
# Schraudolph exp: bf16 bitcast of int16(x*128/ln2 + (127<<7) - C)
SCH_A = float(128.0 / np.log(2.0) * SCALE)
SCH_B = float(127 * 128 - 4)

# which j-slots of each 16-slot phase the DVE handles exp for (rest: ACT)
DVE_EXP_SLOTS = {1, 3, 5, 7, 9, 11, 13, 15}
AV_LAG = 6
TT_DELAY = 3    # slots between finish(recip+bcast) and the OT multiplies


def build_nc(finalize=True):
    nc = bacc.Bacc("TRN2", target_bir_lowering=False)

    xT_d = nc.dram_tensor("xT", [P, KO, N], BF16, kind="ExternalInput")
    Wq_d = nc.dram_tensor("Wq", [P, KO, 256], BF16, kind="ExternalInput")
    Wk_d = nc.dram_tensor("Wk", [P, KO, 256], BF16, kind="ExternalInput")
    Wv_d = nc.dram_tensor("Wv", [P, KO, 256], BF16, kind="ExternalInput")
    Wo_d = nc.dram_tensor("Wo", [P, 2, D], BF16, kind="ExternalInput")
    out_d = nc.dram_tensor("out", [N, D], F32, kind="ExternalOutput")

    with tile.TileContext(nc) as tc, ExitStack() as ctx:
        consts = ctx.enter_context(tc.tile_pool(name="consts", bufs=1))
        # ONE shared rotating PSUM pool: S pairs + q/k/v/o projection units.
        # 3 buffers x [128, 2, 512] f32 = 6 banks.
        big = ctx.enter_context(tc.tile_pool(name="big", bufs=3, space="PSUM"))
        # AV pair accumulator: [65, 2, 512] = 2 banks, single buffer.
        avp = ctx.enter_context(tc.tile_pool(name="avp", bufs=1, space="PSUM"))
        expp = ctx.enter_context(tc.tile_pool(name="expp", bufs=10))
        small = ctx.enter_context(tc.tile_pool(name="small", bufs=4))
        outp = ctx.enter_context(tc.tile_pool(name="outp", bufs=3))

        # ---- persistent SBUF tensors ----
        xT_sb = consts.tile([P, KO, N], BF16, tag="xT")
        Wq_sb = consts.tile([P, KO, 256], BF16, tag="Wq")
        Wk_sb = consts.tile([P, KO, 256], BF16, tag="Wk")
        Wv_sb = consts.tile([P, KO, 256], BF16, tag="Wv")
        Wo_sb = consts.tile([P, 2, D], BF16, tag="Wo")
        QT_sb = consts.tile([P, 2, N], BF16, tag="QT")
        KT_sb = consts.tile([P, 2, N], BF16, tag="KT")
        Vaug_sb = consts.tile([P, JT, HL, DH + 1], BF16, tag="Vaug")
        OT_sb = consts.tile([P, 2, N], BF16, tag="OT")

        # input DMAs ordered so the q/k prelude can start after ~1.5MB
        nc.sync.dma_start(Wq_sb[:], Wq_d[:])
        nc.sync.dma_start(xT_sb[:, :, 0:512], xT_d[:, :, 0:512])
        nc.sync.dma_start(xT_sb[:, :, 512:1024], xT_d[:, :, 512:1024])
        nc.sync.dma_start(Wk_sb[:], Wk_d[:])
        nc.sync.dma_start(Wv_sb[:], Wv_d[:])
        nc.sync.dma_start(xT_sb[:, :, 1024:1536], xT_d[:, :, 1024:1536])
        nc.sync.dma_start(xT_sb[:, :, 1536:2048], xT_d[:, :, 1536:2048])
        nc.sync.dma_start(Wo_sb[:], Wo_d[:])

        nc.vector.memset(Vaug_sb[:, :, :, DH : DH + 1], 1.0)

        # spin the PE so HAM unthrottles before the first real matmuls
        junk = small.tile([64, 64], BF16, tag="junk")
        nc.vector.memset(junk[:], 0.0)
        wp = big.tile([P, 2, 512], F32, tag="big", name="warm")
        for _ in range(56):
            nc.tensor.matmul(wp[0:64, 0, 0:64], lhsT=junk[:], rhs=junk[:],
                             start=True, stop=True)
        # touch the exp table early so ACT_TABLE_LOAD overlaps the DMAs
        warm = small.tile([1, 8], F32, tag="warm")
        nc.scalar.activation(warm[:], junk[0:1, 0:8],
                             mybir.ActivationFunctionType.Exp)

        # ---- projection units (two 512-blocks per unit, k-outer so the
        # stationary weight chunk serves both blocks back-to-back) ----
        def proj_unit(W_sb, T_sb, o, np_, dve_cast=False):
            pp = big.tile([P, 2, 512], F32, tag="big",
                          name=f"pp{id(W_sb)%97}_{o}_{np_}")
            for k in range(KO):
                for m in range(2):
                    nc.tensor.matmul(
                        pp[:, m, :],
                        lhsT=W_sb[:, k, ts(o, P)],
                        rhs=xT_sb[:, k, ds(np_ * 1024 + m * 512, 512)],
                        start=(k == 0),
                        stop=(k == KO - 1),
                    )
            if dve_cast:
                nc.vector.tensor_copy(T_sb[:, o, ds(np_ * 1024, 1024)], pp[:])
            else:
                nc.scalar.activation(T_sb[:, o, ds(np_ * 1024, 1024)], pp[:],
                                     mybir.ActivationFunctionType.Copy)

        def q_unit(o, np_, dve_cast=False):
            proj_unit(Wq_sb, QT_sb, o, np_, dve_cast)

        def k_unit(o, np_, dve_cast=False):
            proj_unit(Wk_sb, KT_sb, o, np_, dve_cast)

        def v_unit(jj, dve_cast=False):
            # two kv j-tiles (256 output cols each) in one PSUM buffer
            vp = big.tile([P, 2, 512], F32, tag="big", name=f"vp{jj}")
            for m in range(2):
                for k in range(KO):
                    nc.tensor.matmul(
                        vp[:, m, 0:256],
                        lhsT=xT_sb[:, k, ds((2 * jj + m) * P, P)],
                        rhs=Wv_sb[:, k, :],
                        start=(k == 0),
                        stop=(k == KO - 1),
                    )
            dst = Vaug_sb[:, 2 * jj : 2 * jj + 2, :, 0:DH]
            src = vp[:, :, 0:256].rearrange("p m (h d) -> p m h d", h=HL)
            if dve_cast:
                nc.vector.tensor_copy(dst, src)
            else:
                nc.scalar.activation(dst, src,
                                     mybir.ActivationFunctionType.Copy)

        # ---- attention stream state ----
        av_tiles = {}     # (i, n) -> psum pair accumulator [65, 2, 512]
        exp_tiles = {}    # (i, n, j) -> E tile [128, 2, 512] bf16

        def s_exp(i, n, j):
            """Score pair (heads 2i, 2i+1) + exp for q block n, kv tile j."""
            sp = big.tile([P, 2, 512], F32, tag="big", name=f"sp{i}_{n}_{j}")
            nc.tensor.matmul(
                sp[:, 0, :],
                lhsT=KT_sb[0:64, i, ts(j, P)],
                rhs=QT_sb[0:64, i, ts(n, 512)],
                start=True, stop=True,
                tile_position=(0, 0),
            )
            nc.tensor.matmul(
                sp[:, 1, :],
                lhsT=KT_sb[64:128, i, ts(j, P)],
                rhs=QT_sb[64:128, i, ts(n, 512)],
                start=True, stop=True,
                tile_position=(64, 0),
            )
            eb = expp.tile([P, 2, 512], BF16, tag="expS", name=f"eb{i}_{n}_{j}")
            if j in DVE_EXP_SLOTS:
                nc.vector.tensor_scalar(
                    eb[:].bitcast(I16), sp[:], SCH_A, SCH_B,
                    mybir.AluOpType.mult, mybir.AluOpType.add,
                )
            else:
                nc.scalar.activation(
                    eb[:], sp[:], mybir.ActivationFunctionType.Exp,
                    scale=SCALE,
                )
            exp_tiles[(i, n, j)] = eb

        def av(i, n, j, s):
            """Accumulate [V|1]^T E for head 2i+s into the pair PSUM tile."""
            eb = exp_tiles[(i, n, j)]
            h = 2 * i + s
            if j == 0 and s == 0:
                av_tiles[(i, n)] = avp.tile(
                    [DH + 1, 2, 512], F32, tag="avp", name=f"av{i}_{n}"
                )
            nc.tensor.matmul(
                av_tiles[(i, n)][:, s, :],
                lhsT=Vaug_sb[:, j, h, :],
                rhs=eb[:, s, :],
                start=(j == 0),
                stop=(j == JT - 1),
                skip_group_check=True,
            )
            if s == 1:
                del exp_tiles[(i, n, j)]

        def av_finish_a(i, n):
            """Pair-merged denominator chain: one row-copy, one recip, one
            broadcast for BOTH heads (row 64 of each bank holds the
            ones-column output = softmax denominator).  partition_broadcast
            can only source physical partition 0, hence the ACT copy; PSUM
            reads may start at partition 64 so the copy is legal.
            """
            acc = av_tiles[(i, n)]
            rs = small.tile([1, 2, 512], F32, tag="rs", name=f"rs{i}_{n}")
            nc.scalar.activation(rs[:], acc[DH : DH + 1, :, :],
                                 mybir.ActivationFunctionType.Copy)
            rc = small.tile([1, 2, 512], F32, tag="rc", name=f"rc{i}_{n}")
            nc.vector.reciprocal_approx_fast(rc[:], rs[:])
            rb = small.tile([DH, 2, 512], F32, tag="rb", name=f"rb{i}_{n}")
            nc.gpsimd.partition_broadcast(rb[:], rc[:])
            return rb

        def av_finish_b(i, n, rb):
            """OT multiplies for the pair; releases the AV PSUM tile."""
            acc = av_tiles.pop((i, n))
            for s in range(2):
                nc.vector.tensor_mul(
                    OT_sb[64 * s : 64 * s + DH, i, ts(n, 512)],
                    acc[0:DH, s, :],
                    rb[:, s, :],
                )

        out_r = out_d.rearrange("(t p) d -> p t d", p=P)

        def o_unit(t, dve_copy=False):
            """Output projection for q tile t (128 rows) + DMA out."""
            op = big.tile([P, 2, 512], F32, tag="big", name=f"op{t}")
            for c in range(2):
                nc.tensor.matmul(
                    op[:, 0, :],
                    lhsT=OT_sb[:, c, ts(t, P)],
                    rhs=Wo_sb[:, c, :],
                    start=(c == 0),
                    stop=(c == 1),
                )
            ot = outp.tile([P, D], F32, tag="out")
            if dve_copy:
                nc.vector.tensor_copy(ot[:], op[:, 0, :])
            else:
                nc.scalar.activation(ot[:], op[:, 0, :],
                                     mybir.ActivationFunctionType.Copy)
            nc.sync.dma_start(out_r[:, t, :], ot[:])

        # ---- schedule ----
        extras = {}

        def addx(g, fn):
            extras.setdefault(g, []).append(fn)

        phases = [(i, n) for i in range(2) for n in range(NB)]
        total = len(phases) * JT

        # phase (0,0): V units at even slots, K second half at slot 3
        for jj in range(8):
            addx(2 * jj, lambda jj=jj, e=(jj % 2 == 1): v_unit(jj, dve_cast=e))
        addx(3, lambda: k_unit(0, 1))
        # phase (0,1): Q chunk 0 second pair, chunk 1 starts
        addx(16 + 1, lambda: q_unit(0, 1))
        addx(16 + 7, lambda: q_unit(1, 0))
        addx(16 + 13, lambda: k_unit(1, 0))
        # phase (0,2): chunk 1 rest
        addx(32 + 3, lambda: q_unit(1, 1))
        addx(32 + 9, lambda: k_unit(1, 1))

        # O-projection of q block m during pair-1 phase m+1 (needs
        # finish(0,m) [pair-0 phases] and finish(1,m) [previous phase]).
        for m in range(NB - 1):
            pbase = (4 + m + 1) * JT
            for t in range(4):
                addx(pbase + 9 + 2 * t,
                     lambda t=t, m=m, e=(t % 2 == 1):
                         o_unit(4 * m + t, dve_copy=e))

        # prelude: minimum to start the stream
        q_unit(0, 0, dve_cast=True)
        k_unit(0, 0, dve_cast=False)

        # flat stream: S pairs flow continuously, AV lags AV_LAG slots.
        for g in range(total + AV_LAG + TT_DELAY + 1):
            ag = g - AV_LAG
            if 0 <= ag < total:
                ia, na = phases[ag // JT]
                av(ia, na, ag % JT, 0)
            if g < total:
                i, n = phases[g // JT]
                s_exp(i, n, g % JT)
            if 0 <= ag < total:
                av(ia, na, ag % JT, 1)
                if ag % JT == JT - 1:
                    rb = av_finish_a(ia, na)
                    addx(g + TT_DELAY,
                         lambda ia=ia, na=na, rb=rb: av_finish_b(ia, na, rb))
            for fn in extras.get(g, []):
                fn()

        # tail: block 3 output projections
        for t in range(4):
            o_unit(12 + t, dve_copy=(t % 2 == 1))

    if finalize:
        nc.finalize()
    return nc


_NC_CACHE = None


def _get_nc():
    global _NC_CACHE
    if _NC_CACHE is None:
        _NC_CACHE = build_nc()
    return _NC_CACHE


def _chunked(w):
    """[512, M] -> [128, 4, M] with row r at [r % 128, r // 128]."""
    m = w.shape[1]
    return np.ascontiguousarray(
        w.reshape(w.shape[0] // P, P, m).transpose(1, 0, 2)
    )


def make_in_maps(x, Wq, Wkv, Wo, bo):
    bf = ml_dtypes.bfloat16
    Wq = np.asarray(Wq, np.float32)
    Wkv = np.asarray(Wkv, np.float32)
    Wo = np.asarray(Wo, np.float32)
    xTs = []
    for b in range(4):
        xTs.append(_chunked(np.asarray(x[b], np.float32).T).astype(bf))
    in_maps = []
    for c in range(8):
        b, hh = divmod(c, 2)
        cols = slice(hh * 256, (hh + 1) * 256)
        in_maps.append({
            "xT": xTs[b],
            "Wq": _chunked(Wq[:, cols]).astype(bf),
            "Wk": _chunked(Wkv[:, :D][:, cols]).astype(bf),
            "Wv": _chunked(Wkv[:, D:][:, cols]).astype(bf),
            "Wo": _chunked(Wo[hh * 256 : (hh + 1) * 256, :]).astype(bf),
        })
    return in_maps


def gather_out(results, x, bo):
    b_total = x.shape[0]
    bo = np.asarray(bo, np.float32)
    out = np.empty((b_total, N, D), np.float32)
    for b in range(b_total):
        out[b] = results[2 * b]["out"] + results[2 * b + 1]["out"] + bo
    return out


def kernel(x, Wq, Wkv, Wo, bo, trace=False):
    nc = _get_nc()
    in_maps = make_in_maps(x, Wq, Wkv, Wo, bo)
    res = run_bass_kernel_spmd(nc, in_maps, core_ids=list(range(8)), trace=trace)
    out = gather_out(res.results, np.asarray(x), bo)
    if trace:
        kernel.last_exec_time_ns = res.exec_time_ns
    return out


kernel.last_exec_time_ns = None


# revision 13
# speedup vs baseline: 1.2141x; 1.1011x over previous
"""Multi-head self-attention (b=4, n=2048, d=512, h=8, dh=64) on 8 trn2 cores.

Sharding: core c -> (batch b = c//2, head half hh = c%2). Each core computes
4 heads over the FULL sequence; the two cores of a batch produce partial
outputs (their heads' slice of the output projection) that the host SUMS,
adding the bias.

v3 schedule notes (vs v2 baseline):
  - ONE shared 3-buffer PSUM pool (6 banks) rotates S-score pairs AND all
    projection units, so the S stream runs 3 deep: S(g+3) only WARs on
    exp(g), letting the two exp engines (ACT true-exp / DVE Schraudolph)
    overlap fully instead of serializing the slot cadence.
  - AV accumulators for the head pair live in ONE [65, 2, 512] PSUM tile
    (2 banks).  Normalize is pair-batched: reciprocal_approx_fast reads the
    whole tile straight from PSUM (rows 0..63 produce garbage recips that
    are never read - only row 64, the ones-row denominator, is used), so
    the old per-head row-64 ACT extraction copies are gone.
  - The denominator broadcast runs on GpSimd; the two OT multiplies are
    emitted a few slots later so they never head-of-line block DVE exps.
  - Input DMA is reordered (Wq, xT[0:1024], Wk, ...) so the Q/K prelude
    projections start ~5us in instead of waiting for the full 2.8MB.
  - AV lags the S stream by 6 slots; the tail drains in ~10us.
"""

import sys

sys.path.insert(0, "/opt/trn_rl_repo")

from contextlib import ExitStack

import ml_dtypes
import numpy as np

import concourse.bass as bass
import concourse.tile as tile
from concourse import bacc, mybir
from concourse.bass import ts, ds
from concourse.bass_utils import run_bass_kernel_spmd

BF16 = mybir.dt.bfloat16
F32 = mybir.dt.float32
I16 = mybir.dt.int16

D = 512         # model dim
HL = 4          # heads per core
DH = 64
N = 2048        # full sequence per core
P = 128
KO = 4          # xT chunks of model dim
JT = 16         # kv tiles of 128
NB = 4          # q blocks of 512
SCALE = DH ** -0.5
# Schraudolph exp: bf16 bitcast of int16(x*128/ln2 + (127<<7) - C)
SCH_A = float(128.0 / np.log(2.0) * SCALE)
SCH_B = float(127 * 128 - 4)

# which j-slots of each 16-slot phase the DVE handles exp for (rest: ACT)
DVE_EXP_SLOTS = {1, 3, 5, 7, 9, 11, 13, 15}
AV_LAG = 6
TT_DELAY = 3    # slots between finish(recip+bcast) and the OT multiplies


def build_nc(finalize=True):
    nc = bacc.Bacc("TRN2", target_bir_lowering=False)

    xT_d = nc.dram_tensor("xT", [P, KO, N], BF16, kind="ExternalInput")
    Wq_d = nc.dram_tensor("Wq", [P, KO, 256], BF16, kind="ExternalInput")
    Wk_d = nc.dram_tensor("Wk", [P, KO, 256], BF16, kind="ExternalInput")
    Wv_d = nc.dram_tensor("Wv", [P, KO, 256], BF16, kind="ExternalInput")
    Wo_d = nc.dram_tensor("Wo", [P, 2, D], BF16, kind="ExternalInput")
    out_d = nc.dram_tensor("out", [N, D], F32, kind="ExternalOutput")

    with tile.TileContext(nc) as tc, ExitStack() as ctx:
        consts = ctx.enter_context(tc.tile_pool(name="consts", bufs=1))
        # ONE shared rotating PSUM pool: S pairs + q/k/v/o projection units.
        # 3 buffers x [128, 2, 512] f32 = 6 banks.
        big = ctx.enter_context(tc.tile_pool(name="big", bufs=3, space="PSUM"))
        # AV pair accumulator: [65, 2, 512] = 2 banks, single buffer.
        avp = ctx.enter_context(tc.tile_pool(name="avp", bufs=1, space="PSUM"))
        expp = ctx.enter_context(tc.tile_pool(name="expp", bufs=16))
        small = ctx.enter_context(tc.tile_pool(name="small", bufs=4))
        outp = ctx.enter_context(tc.tile_pool(name="outp", bufs=3))

        # ---- persistent SBUF tensors ----
        xT_sb = consts.tile([P, KO, N], BF16, tag="xT")
        Wq_sb = consts.tile([P, KO, 256], BF16, tag="Wq")
        Wk_sb = consts.tile([P, KO, 256], BF16, tag="Wk")
        Wv_sb = consts.tile([P, KO, 256], BF16, tag="Wv")
        Wo_sb = consts.tile([P, 2, D], BF16, tag="Wo")
        QT_sb = consts.tile([P, 2, N], BF16, tag="QT")
        KT_sb = consts.tile([P, 2, N], BF16, tag="KT")
        Vaug_sb = consts.tile([P, JT, HL, DH + 1], BF16, tag="Vaug")
        OT_sb = consts.tile([P, 2, N], BF16, tag="OT")

        # input DMAs ordered so the q/k prelude can start after ~1.5MB
        nc.sync.dma_start(Wq_sb[:], Wq_d[:])
        nc.sync.dma_start(xT_sb[:, :, 0:512], xT_d[:, :, 0:512])
        nc.sync.dma_start(xT_sb[:, :, 512:1024], xT_d[:, :, 512:1024])
        nc.sync.dma_start(Wk_sb[:], Wk_d[:])
        nc.sync.dma_start(Wv_sb[:], Wv_d[:])
        nc.sync.dma_start(xT_sb[:, :, 1024:1536], xT_d[:, :, 1024:1536])
        nc.sync.dma_start(xT_sb[:, :, 1536:2048], xT_d[:, :, 1536:2048])
        nc.sync.dma_start(Wo_sb[:], Wo_d[:])

        nc.vector.memset(Vaug_sb[:, :, :, DH : DH + 1], 1.0)

        # spin the PE so HAM unthrottles before the first real matmuls
        junk = small.tile([64, 64], BF16, tag="junk")
        nc.vector.memset(junk[:], 0.0)
        wp = big.tile([P, 2, 512], F32, tag="big", name="warm")
        for _ in range(56):
            nc.tensor.matmul(wp[0:64, 0, 0:64], lhsT=junk[:], rhs=junk[:],
                             start=True, stop=True)
        # touch the exp table early so ACT_TABLE_LOAD overlaps the DMAs
        warm = small.tile([1, 8], F32, tag="warm")
        nc.scalar.activation(warm[:], junk[0:1, 0:8],
                             mybir.ActivationFunctionType.Exp)

        # ---- projection units (two 512-blocks per unit, k-outer so the
        # stationary weight chunk serves both blocks back-to-back) ----
        def proj_unit(W_sb, T_sb, o, np_, dve_cast=False):
            pp = big.tile([P, 2, 512], F32, tag="big",
                          name=f"pp{id(W_sb)%97}_{o}_{np_}")
            for k in range(KO):
                for m in range(2):
                    nc.tensor.matmul(
                        pp[:, m, :],
                        lhsT=W_sb[:, k, ts(o, P)],
                        rhs=xT_sb[:, k, ds(np_ * 1024 + m * 512, 512)],
                        start=(k == 0),
                        stop=(k == KO - 1),
                    )
            if dve_cast:
                nc.vector.tensor_copy(T_sb[:, o, ds(np_ * 1024, 1024)], pp[:])
            else:
                nc.scalar.activation(T_sb[:, o, ds(np_ * 1024, 1024)], pp[:],
                                     mybir.ActivationFunctionType.Copy)

        def q_unit(o, np_, dve_cast=False):
            proj_unit(Wq_sb, QT_sb, o, np_, dve_cast)

        def k_unit(o, np_, dve_cast=False):
            proj_unit(Wk_sb, KT_sb, o, np_, dve_cast)

        def proj_part(W_sb, T_sb, o, cb, dve_cast=False):
            """Single 512-col projection part (prelude granularity): only
            needs xT columns [cb*512, cb*512+512), so it can start as soon
            as that input DMA block lands."""
            pp = big.tile([P, 2, 512], F32, tag="big",
                          name=f"pt{id(W_sb)%97}_{o}_{cb}")
            for k in range(KO):
                nc.tensor.matmul(
                    pp[:, 0, :],
                    lhsT=W_sb[:, k, ts(o, P)],
                    rhs=xT_sb[:, k, ts(cb, 512)],
                    start=(k == 0),
                    stop=(k == KO - 1),
                )
            if dve_cast:
                nc.vector.tensor_copy(T_sb[:, o, ts(cb, 512)], pp[:, 0, :])
            else:
                nc.scalar.activation(T_sb[:, o, ts(cb, 512)], pp[:, 0, :],
                                     mybir.ActivationFunctionType.Copy)

        def v_unit(jj, dve_cast=False):
            # two kv j-tiles (256 output cols each) in one PSUM buffer
            vp = big.tile([P, 2, 512], F32, tag="big", name=f"vp{jj}")
            for m in range(2):
                for k in range(KO):
                    nc.tensor.matmul(
                        vp[:, m, 0:256],
                        lhsT=xT_sb[:, k, ds((2 * jj + m) * P, P)],
                        rhs=Wv_sb[:, k, :],
                        start=(k == 0),
                        stop=(k == KO - 1),
                    )
            dst = Vaug_sb[:, 2 * jj : 2 * jj + 2, :, 0:DH]
            src = vp[:, :, 0:256].rearrange("p m (h d) -> p m h d", h=HL)
            if dve_cast:
                nc.vector.tensor_copy(dst, src)
            else:
                nc.scalar.activation(dst, src,
                                     mybir.ActivationFunctionType.Copy)

        # ---- attention stream state ----
        av_tiles = {}     # (i, n) -> psum pair accumulator [65, 2, 512]
        exp_tiles = {}    # (i, n, j) -> E tile [128, 2, 512] bf16

        def s_exp(i, n, j):
            """Score pair (heads 2i, 2i+1) + exp for q block n, kv tile j."""
            sp = big.tile([P, 2, 512], F32, tag="big", name=f"sp{i}_{n}_{j}")
            nc.tensor.matmul(
                sp[:, 0, :],
                lhsT=KT_sb[0:64, i, ts(j, P)],
                rhs=QT_sb[0:64, i, ts(n, 512)],
                start=True, stop=True,
                tile_position=(0, 0),
            )
            nc.tensor.matmul(
                sp[:, 1, :],
                lhsT=KT_sb[64:128, i, ts(j, P)],
                rhs=QT_sb[64:128, i, ts(n, 512)],
                start=True, stop=True,
                tile_position=(64, 0),
            )
            eb = expp.tile([P, 2, 512], BF16, tag="expS", name=f"eb{i}_{n}_{j}")
            if j in DVE_EXP_SLOTS:
                nc.vector.tensor_scalar(
                    eb[:].bitcast(I16), sp[:], SCH_A, SCH_B,
                    mybir.AluOpType.mult, mybir.AluOpType.add,
                )
            else:
                nc.scalar.activation(
                    eb[:], sp[:], mybir.ActivationFunctionType.Exp,
                    scale=SCALE,
                )
            exp_tiles[(i, n, j)] = eb

        def av(i, n, j, s):
            """Accumulate [V|1]^T E for head 2i+s into the pair PSUM tile."""
            eb = exp_tiles[(i, n, j)]
            h = 2 * i + s
            if j == 0 and s == 0:
                av_tiles[(i, n)] = avp.tile(
                    [DH + 1, 2, 512], F32, tag="avp", name=f"av{i}_{n}"
                )
            nc.tensor.matmul(
                av_tiles[(i, n)][:, s, :],
                lhsT=Vaug_sb[:, j, h, :],
                rhs=eb[:, s, :],
                start=(j == 0),
                stop=(j == JT - 1),
                skip_group_check=True,
            )
            if s == 1:
                del exp_tiles[(i, n, j)]

        def av_finish_a(i, n):
            """Denominator chain + early AV PSUM release.

            Row 64 of each bank holds the ones-column output (softmax
            denominator).  One ACT row-copy (partition_broadcast can only
            source physical partition 0; PSUM reads may start at partition
            64 so the copy is legal), one pair recip, one pair broadcast.
            The AV values are evacuated to SBUF immediately (split across
            ACT and DVE) so the single avp PSUM buffer frees ~2us after
            the last AV matmul instead of after the whole chain - the next
            phase's AV stream stalls far less.
            """
            acc = av_tiles.pop((i, n))
            rs = small.tile([1, 2, 512], F32, tag="rs", name=f"rs{i}_{n}")
            nc.scalar.activation(rs[:], acc[DH : DH + 1, :, :],
                                 mybir.ActivationFunctionType.Copy)
            accS = small.tile([DH, 2, 512], BF16, tag="accS",
                              name=f"accS{i}_{n}")
            nc.scalar.activation(accS[:, 0, :], acc[0:DH, 0, :],
                                 mybir.ActivationFunctionType.Copy)
            nc.vector.tensor_copy(accS[:, 1, :], acc[0:DH, 1, :])
            rc = small.tile([1, 2, 512], F32, tag="rc", name=f"rc{i}_{n}")
            nc.vector.reciprocal_approx_fast(rc[:], rs[:])
            rb = small.tile([DH, 2, 512], F32, tag="rb", name=f"rb{i}_{n}")
            nc.gpsimd.partition_broadcast(rb[:], rc[:])
            return accS, rb

        def av_finish_b(i, n, accS, rb):
            """OT multiplies for the pair (from the SBUF copy)."""
            for s in range(2):
                nc.vector.tensor_mul(
                    OT_sb[64 * s : 64 * s + DH, i, ts(n, 512)],
                    accS[:, s, :],
                    rb[:, s, :],
                )

        out_r = out_d.rearrange("(t p) d -> p t d", p=P)

        def o_unit(t, dve_copy=False):
            """Output projection for q tile t (128 rows) + DMA out."""
            op = big.tile([P, 2, 512], F32, tag="big", name=f"op{t}")
            for c in range(2):
                nc.tensor.matmul(
                    op[:, 0, :],
                    lhsT=OT_sb[:, c, ts(t, P)],
                    rhs=Wo_sb[:, c, :],
                    start=(c == 0),
                    stop=(c == 1),
                )
            ot = outp.tile([P, D], F32, tag="out")
            if dve_copy:
                nc.vector.tensor_copy(ot[:], op[:, 0, :])
            else:
                nc.scalar.activation(ot[:], op[:, 0, :],
                                     mybir.ActivationFunctionType.Copy)
            nc.sync.dma_start(out_r[:, t, :], ot[:])

        # ---- schedule ----
        extras = {}

        def addx(g, fn):
            extras.setdefault(g, []).append(fn)

        phases = [(i, n) for i in range(2) for n in range(NB)]
        total = len(phases) * JT

        # phase (0,0): V units at odd slots; K/Q 512-col parts at even
        # slots, ordered by when the S/AV streams need them (K cols c*512
        # feed S slots j = 4c..4c+3).
        for jj in range(8):
            addx(2 * jj + 1,
                 lambda jj=jj, e=(jj % 2 == 1): v_unit(jj, dve_cast=e))
        addx(0, lambda: proj_part(Wk_sb, KT_sb, 0, 1))
        addx(2, lambda: proj_part(Wk_sb, KT_sb, 0, 2))
        addx(4, lambda: proj_part(Wk_sb, KT_sb, 0, 3))
        addx(6, lambda: proj_part(Wq_sb, QT_sb, 0, 1))
        # phase (0,1): Q chunk 0 second pair, chunk 1 starts
        addx(16 + 1, lambda: q_unit(0, 1))
        addx(16 + 7, lambda: q_unit(1, 0))
        addx(16 + 13, lambda: k_unit(1, 0))
        # phase (0,2): chunk 1 rest
        addx(32 + 3, lambda: q_unit(1, 1))
        addx(32 + 9, lambda: k_unit(1, 1))

        # O-projection of q block m during pair-1 phase m+1 (needs
        # finish(0,m) [pair-0 phases] and finish(1,m) [previous phase]).
        for m in range(NB - 1):
            pbase = (4 + m + 1) * JT
            for t in range(4):
                addx(pbase + 9 + 2 * t,
                     lambda t=t, m=m, e=(t % 2 == 1):
                         o_unit(4 * m + t, dve_copy=e))

        # prelude: minimum to start the stream (first q/k 512-col parts
        # only need xT cols 0:512 = the first input DMA block)
        proj_part(Wq_sb, QT_sb, 0, 0, dve_cast=True)
        proj_part(Wk_sb, KT_sb, 0, 0, dve_cast=False)

        # flat stream: S pairs flow continuously, AV lags AV_LAG slots.
        for g in range(total + AV_LAG + TT_DELAY + 1):
            ag = g - AV_LAG
            if 0 <= ag < total:
                ia, na = phases[ag // JT]
                av(ia, na, ag % JT, 0)
            if g < total:
                i, n = phases[g // JT]
                s_exp(i, n, g % JT)
            if 0 <= ag < total:
                av(ia, na, ag % JT, 1)
                if ag % JT == JT - 1:
                    accS, rb = av_finish_a(ia, na)
                    dly = 1 if ag == total - 1 else TT_DELAY
                    addx(g + dly,
                         lambda ia=ia, na=na, accS=accS, rb=rb:
                             av_finish_b(ia, na, accS, rb))
            for fn in extras.get(g, []):
                fn()

        # tail: block 3 output projections
        for t in range(4):
            o_unit(12 + t, dve_copy=(t % 2 == 1))

    if finalize:
        nc.finalize()
    return nc


_NC_CACHE = None


def _get_nc():
    global _NC_CACHE
    if _NC_CACHE is None:
        _NC_CACHE = build_nc()
    return _NC_CACHE


def _chunked(w):
    """[512, M] -> [128, 4, M] with row r at [r % 128, r // 128]."""
    m = w.shape[1]
    return np.ascontiguousarray(
        w.reshape(w.shape[0] // P, P, m).transpose(1, 0, 2)
    )


def make_in_maps(x, Wq, Wkv, Wo, bo):
    bf = ml_dtypes.bfloat16
    Wq = np.asarray(Wq, np.float32)
    Wkv = np.asarray(Wkv, np.float32)
    Wo = np.asarray(Wo, np.float32)
    xTs = []
    for b in range(4):
        xTs.append(_chunked(np.asarray(x[b], np.float32).T).astype(bf))
    in_maps = []
    for c in range(8):
        b, hh = divmod(c, 2)
        cols = slice(hh * 256, (hh + 1) * 256)
        in_maps.append({
            "xT": xTs[b],
            "Wq": _chunked(Wq[:, cols]).astype(bf),
            "Wk": _chunked(Wkv[:, :D][:, cols]).astype(bf),
            "Wv": _chunked(Wkv[:, D:][:, cols]).astype(bf),
            "Wo": _chunked(Wo[hh * 256 : (hh + 1) * 256, :]).astype(bf),
        })
    return in_maps


def gather_out(results, x, bo):
    b_total = x.shape[0]
    bo = np.asarray(bo, np.float32)
    out = np.empty((b_total, N, D), np.float32)
    for b in range(b_total):
        out[b] = results[2 * b]["out"] + results[2 * b + 1]["out"] + bo
    return out


def kernel(x, Wq, Wkv, Wo, bo, trace=False):
    nc = _get_nc()
    in_maps = make_in_maps(x, Wq, Wkv, Wo, bo)
    res = run_bass_kernel_spmd(nc, in_maps, core_ids=list(range(8)), trace=trace)
    out = gather_out(res.results, np.asarray(x), bo)
    if trace:
        kernel.last_exec_time_ns = res.exec_time_ns
    return out


kernel.last_exec_time_ns = None
